# revision 21
# baseline (speedup 1.0000x reference)
"""Gabor-modulated conv-weight synthesis on 8 Trainium2 NeuronCores.

Computes out[g*CO + co, ci, h, w] = gabor(theta[g], lam[g])[h, w] * x[co, ci, h, w]
for x: [512, 512, 9, 9] f32, theta/lam: [4] f32  ->  out: [2048, 512, 9, 9] f32.

Sharding: x along C_out into 8 shards of 64; theta/lam replicated; each core
produces its [4, 64, 512, 9, 9] output slice with no communication.

Per-core device program (Bass/Tile), DMA-engine-time bound (26.7 MB of
HBM traffic per core through 16 DMA engines at ~26.6 GB/s each):
  - one step-0 broadcast DMA loads grids+theta+lam to all 128 partitions
    as the FIRST descriptor batch on the sync HWDGE ring (ring FIFO puts
    it ahead of the x packets, so no cross-queue starvation and no gpsimd
    partition_broadcast, which would serialize behind a SWDGE feed),
  - synthesize the 4 Gabor filters [4, 81] on-device from theta/lam with a
    short ACT/DVE chain (cos th = sin(pi/2 - th) via ACT bias; cos 2pi t =
    1 - 2 sin^2(pi (t - k)) with k from an int32 round-trip, valid for any
    cast rounding mode),
  - keep ALL of x resident in SBUF (41.5 KB/partition in bf16), loads
    queued up-front across both HWDGE rings in [32, 96, 128]-row chunks
    (small first chunk so the const packets are not starved); the first
    store issues ~20 us in while loads keep the DMA engines fed,
  - the output is g-pair interleaved in DRAM ([q, rows, j, hw], g = 2q+j)
    so each store covers two filters with 20.7 KB per-partition runs (the
    DMA engines' best packet size) while the DVE multiplies stay at the
    64-row tile size where the 16-bit 2x fast path is fastest
    (~233 G elem/s); two muls fill one pair tile, stores alternate the
    two HWDGE rings (SP and ACT); the host un-interleaves with a reshape.

The kernel is DMA-bound, so x is streamed and the output written in
bfloat16 (the filters stay f32 on-device; the host casts the result back
to f32). That halves HBM traffic; the bf16 rounding costs ~4e-3 max rel
error against the f32 reference, well under the 2e-2 gate.
"""

import ml_dtypes
import numpy as np

import concourse.bass as bass
import concourse.bacc as bacc
import concourse.mybir as mybir
from concourse.tile import TileContext
from concourse.bass_utils import run_bass_kernel_spmd

N_CORES = 8
G = 4
CO, CI, H, W = 512, 512, 9, 9
HW = H * W                # 81
CO_SH = CO // N_CORES     # 64 C_out rows per core
ROWS = CO_SH * CI         # 32768 (co_local, ci) rows per core
P = 128                   # SBUF partitions
NPP = ROWS // P           # 256 rows per partition
SIGMA = float(np.pi)      # Gaussian envelope std of the Gabor synthesis

# all stores cover 64 rows x a g-pair: every packet is a 20.7 KB
# per-partition run, the DMA engines' best size. The first block's muls
# are split 32+32 so they nest inside the load-chunk boundaries
# (cumulative 32/128/256) and the store stream still starts early.
NS = 64
SEGS = {0: [(0, 32), (32, 32)], 64: [(64, 64)],
        128: [(128, 64)], 192: [(192, 64)]}
NSMAX = NS

F32 = mybir.dt.float32
BF16 = mybir.dt.bfloat16
I32 = mybir.dt.int32
AF = mybir.ActivationFunctionType
ALU = mybir.AluOpType

NCONST = 3 * HW + 2 * G   # x-grid, y-grid, envelope, theta, lam


def build_bass(rows=ROWS):

    nc = bacc.Bacc("TRN2", target_bir_lowering=False, debug=False)
    x = nc.declare_dram_parameter("x", [rows, HW], BF16, isOutput=False)
    # cst[0:81]=x-grid, [81:162]=y-grid, [162:243]=envelope,
    # [243:247]=theta, [247:251]=lam
    cst = nc.declare_dram_parameter("cst", [NCONST], F32, isOutput=False)
    # g-pair-interleaved output: out[q, r, j, hw] holds filter g = 2q + j,
    # so one store covers two filters with 2x-long contiguous runs
    out = nc.declare_dram_parameter("out", [2, rows, 2, HW], BF16,
                                    isOutput=True)

    xv = x.ap().rearrange("(p n) m -> p n m", p=P)                 # [128, npp, 81]
    ov = out.ap().rearrange("q (p n) j m -> p q n j m", p=P)

    with TileContext(nc) as tc:
        with tc.tile_pool(name="consts", bufs=1) as cpool, \
             tc.tile_pool(name="xs", bufs=1) as xpool, \
             tc.tile_pool(name="outs", bufs=6) as opool:
            # ---- const broadcast-load: first descriptors on the sync
            # HWDGE ring, so the 128 tiny step-0 packets drain before any
            # x packet and the grids are on-chip by ~9 us ----
            cb = cpool.tile([P, NCONST], F32)
            nc.sync.dma_start(cb, cst.ap().unsqueeze(0).broadcast_to([P, NCONST]))

            # ---- x loads, all queued now, all resident, alternating the
            # two HWDGE rings (no SWDGE: two queues total keeps the DMA
            # engines out of 3-way arbitration, and ring FIFO sequences
            # loads ahead of stores with zero engine idle) ----
            # first load chunk stays small so the const packets are not
            # starved behind 20.7 KB load packets at the engines
            LOADS = [32, 96, 128]
            xts = []  # (tile, row0, nrows)
            r0 = 0
            for i, nr in enumerate(LOADS):
                xt = xpool.tile([P, nr * HW], BF16, tag=f"x{i}", bufs=1,
                                name=f"xt{i}")
                eng = nc.scalar if i % 2 == 0 else nc.sync
                eng.dma_start(xt, xv[:, r0:r0 + nr, :])
                xts.append((xt, r0, nr))
                r0 += nr

            def xrows(n0, ns):  # [P, ns, 81] view of rows [n0, n0+ns)
                t, r0, nr = next(e for e in xts
                                 if e[1] <= n0 and n0 + ns <= e[1] + e[2])
                b = (n0 - r0) * HW
                return t[:, b:b + ns * HW].rearrange("p (n m) -> p n m", m=HW)

            xs_t = cb[:, 0:HW]
            ys_t = cb[:, HW:2 * HW]
            env_t = cb[:, 2 * HW:3 * HW]
            th_t = cb[:, 3 * HW:3 * HW + G]
            lm_t = cb[:, 3 * HW + G:3 * HW + 2 * G]

            def per_g(t):  # [128, G] -> [128, G, HW] step-0 view
                return t.unsqueeze(2).broadcast_to([P, G, HW])

            def over_g(ap):  # [128, 81] -> [128, G, 81] step-0 view
                return ap.unsqueeze(1).broadcast_to([P, G, HW])

            # ---- Gabor synthesis, replicated on all 128 partitions ----
            hpi = cpool.tile([P, 1], F32)
            nc.vector.memset(hpi, float(np.pi / 2))  # no DMA dep: runs early
            sin_t = cpool.tile([P, G], F32)
            nc.scalar.activation(sin_t, th_t, AF.Sin)                   # sin th
            cos_t = cpool.tile([P, G], F32)
            # cos th = sin(pi/2 - th); th in [0, 3pi/4] keeps the argument
            # inside ACT Sin's valid [-pi, pi]
            nc.scalar.activation(cos_t, th_t, AF.Sin, scale=-1.0, bias=hpi)

            xr = cpool.tile([P, G, HW], F32)
            t2 = cpool.tile([P, G, HW], F32)
            nc.vector.tensor_mul(xr, over_g(xs_t), per_g(cos_t))
            nc.vector.tensor_mul(t2, over_g(ys_t), per_g(sin_t))
            nc.vector.tensor_add(xr, xr, t2)                            # rotated x
            tt = cpool.tile([P, G, HW], F32)
            nc.vector.tensor_mul(tt, xr, per_g(lm_t))                   # t = xr*lam
            # range-reduce t via int32 round-trip (ACT Sin is only valid on
            # [-pi, pi]; DVE has no mod). Any nearby-integer shift k works:
            # cos(2pi t) = 1 - 2 sin^2(pi (t - k)).
            ti = cpool.tile([P, G, HW], I32)
            nc.vector.tensor_copy(ti, tt)
            tf = cpool.tile([P, G, HW], F32)
            nc.vector.tensor_copy(tf, ti)
            nc.vector.tensor_sub(tt, tt, tf)
            ss = cpool.tile([P, G, HW], F32)
            nc.scalar.activation(ss, tt, AF.Sin, scale=SIGMA)           # sin(pi m)
            gb = cpool.tile([P, G * HW], F32)
            gbg = gb.rearrange("p (g m) -> p g m", m=HW)
            nc.vector.tensor_mul(gbg, ss, ss)
            nc.vector.tensor_scalar(gb, gb, -2.0, 1.0, ALU.mult, ALU.add)  # cos
            gbb = cpool.tile([P, G * HW], BF16)
            gbbg = gbb.rearrange("p (g m) -> p g m", m=HW)
            nc.vector.tensor_mul(gbbg, gbg, over_g(env_t))  # * envelope, to bf16

            def gbv(g, ns):  # filter g broadcast over ns rows (step-0 view)
                return gbb[:, g * HW:(g + 1) * HW].unsqueeze(1).broadcast_to(
                    [P, ns, HW])

            # ---- streaming broadcast-multiply; muls (split along rows
            # where load-chunk boundaries require) fill a g-pair-
            # interleaved tile, one store per (block, pair) alternating
            # the two HWDGE rings (SP and ACT) ----
            for n0 in sorted(SEGS):
                for q in range(2):
                    ot = opool.tile([P, NS * 2 * HW], BF16, tag="o",
                                    name="ot")
                    otv = ot.rearrange("p (n j m) -> p n j m", j=2, m=HW)
                    for j in range(2):
                        for s0, sn in SEGS[n0]:
                            nc.vector.tensor_tensor(
                                otv[:, s0 - n0:s0 - n0 + sn, j, :],
                                xrows(s0, sn), gbv(2 * q + j, sn), ALU.mult)
                    eng = nc.sync if q == 0 else nc.scalar
                    eng.dma_start(ov[:, q, n0:n0 + NS], otv)
    nc.finalize()  # Bacc passes: wait legalization, reg alloc, act table loads
    return nc


def make_const_row(theta, lam):
    ys = np.arange(H, dtype=np.float32) - (H - 1) / 2.0
    xs = np.arange(W, dtype=np.float32) - (W - 1) / 2.0
    y, x = np.meshgrid(ys, xs, indexing="ij")
    env = np.exp(-(x ** 2 + y ** 2) / (2.0 * np.float32(SIGMA) ** 2))
    return np.concatenate(
        [v.reshape(-1) for v in (x, y, env)] + [theta, lam]
    ).astype(np.float32)  # [3 * 81 + 2 * G]


_NC = None
TRACE = False          # set True by the local test harness for NTFF timing
LAST_RESULT = None     # BassKernelResults of the most recent run


def kernel(x, theta, lam):
    global _NC
    if _NC is None:
        _NC = build_bass()
    x = np.asarray(x, dtype=np.float32)
    theta = np.asarray(theta, dtype=np.float32).reshape(G)
    lam = np.asarray(lam, dtype=np.float32).reshape(G)
    cst = make_const_row(theta, lam)
    xb = np.ascontiguousarray(x.astype(ml_dtypes.bfloat16))

    in_maps = []
    for m in range(N_CORES):
        shard = xb[m * CO_SH:(m + 1) * CO_SH].reshape(ROWS, HW)
        in_maps.append({"x": shard, "cst": cst})

    global LAST_RESULT
    LAST_RESULT = run_bass_kernel_spmd(
        _NC, in_maps, list(range(N_CORES)), trace=TRACE
    )
    res = LAST_RESULT.results

    out = np.empty((G, CO, CI, H, W), dtype=np.float32)
    for m in range(N_CORES):
        o = np.asarray(res[m]["out"]).astype(np.float32)
        o = o.reshape(2, ROWS, 2, HW).transpose(0, 2, 1, 3)  # -> [G, rows, hw]
        out[:, m * CO_SH:(m + 1) * CO_SH] = o.reshape(G, CO_SH, CI, H, W)
    return out.reshape(G * CO, CI, H, W)



# revision 22
# speedup vs baseline: 1.2119x; 1.2119x over previous
"""Gabor-modulated conv-weight synthesis on 8 Trainium2 NeuronCores.

Computes out[g*CO + co, ci, h, w] = gabor(theta[g], lam[g])[h, w] * x[co, ci, h, w]
for x: [512, 512, 9, 9] f32, theta/lam: [4] f32  ->  out: [2048, 512, 9, 9] f32.

Sharding: x along C_out into 8 shards of 64; theta/lam replicated; each core
produces its [4, 64, 512, 9, 9] output slice with no communication.

Per-core device program (Bass/Tile), DMA-engine-time bound (26.7 MB of
HBM traffic per core through 16 DMA engines at ~26.6 GB/s each):
  - one step-0 broadcast DMA loads grids+theta+lam to all 128 partitions
    as the FIRST descriptor batch on the sync HWDGE ring (ring FIFO puts
    it ahead of the x packets, so no cross-queue starvation and no gpsimd
    partition_broadcast, which would serialize behind a SWDGE feed),
  - synthesize the 4 Gabor filters [4, 81] on-device from theta/lam with a
    short ACT/DVE chain (cos th = sin(pi/2 - th) via ACT bias; cos 2pi t =
    1 - 2 sin^2(pi (t - k)) with k from an int32 round-trip, valid for any
    cast rounding mode),
  - keep ALL of x resident in SBUF (41.5 KB/partition in bf16), loads
    queued up-front across both HWDGE rings in [32, 96, 128]-row chunks
    (small first chunk so the const packets are not starved); the first
    store issues ~20 us in while loads keep the DMA engines fed,
  - the output is g-pair interleaved in DRAM ([q, rows, j, hw], g = 2q+j)
    so each store covers two filters with 20.7 KB per-partition runs (the
    DMA engines' best packet size) while the DVE multiplies stay at the
    64-row tile size where the 16-bit 2x fast path is fastest
    (~233 G elem/s); two muls fill one pair tile, stores alternate the
    two HWDGE rings (SP and ACT); the host un-interleaves with a reshape.

The kernel is DMA-bound, so x is streamed and the output written in
bfloat16 (the filters stay f32 on-device; the host casts the result back
to f32). That halves HBM traffic; the bf16 rounding costs ~4e-3 max rel
error against the f32 reference, well under the 2e-2 gate.
"""

import ml_dtypes
import numpy as np

import concourse.bass as bass
import concourse.bacc as bacc
import concourse.mybir as mybir
from concourse.tile import TileContext
from concourse.bass_utils import run_bass_kernel_spmd

N_CORES = 8
G = 4
CO, CI, H, W = 512, 512, 9, 9
HW = H * W                # 81
CO_SH = CO // N_CORES     # 64 C_out rows per core
ROWS = CO_SH * CI         # 32768 (co_local, ci) rows per core
P = 128                   # SBUF partitions
NPP = ROWS // P           # 256 rows per partition
SIGMA = float(np.pi)      # Gaussian envelope std of the Gabor synthesis

# muls run per (block, g) at the 64-row size where the DVE 16-bit fast
# path is fastest; stores cover a (block, g-pair) so per-partition runs
# reach 20.7 KB, the DMA engines' best packet size. Block boundaries must
# nest inside the load-chunk boundaries (cumulative 32/128/256).
BLOCKS = [32, 32, 64, 64, 64]
NSMAX = max(BLOCKS)

F32 = mybir.dt.float32
BF16 = mybir.dt.bfloat16
I32 = mybir.dt.int32
AF = mybir.ActivationFunctionType
ALU = mybir.AluOpType

NCONST = 3 * HW + 2 * G   # x-grid, y-grid, envelope, theta, lam


def build_bass(rows=ROWS):

    nc = bacc.Bacc("TRN2", target_bir_lowering=False, debug=False)
    x = nc.declare_dram_parameter("x", [rows, HW], BF16, isOutput=False)
    # cst[0:81]=x-grid, [81:162]=y-grid, [162:243]=envelope,
    # [243:247]=theta, [247:251]=lam
    cst = nc.declare_dram_parameter("cst", [NCONST], F32, isOutput=False)
    # g-pair-interleaved output: out[q, r, j, hw] holds filter g = 2q + j,
    # so one store covers two filters with 2x-long contiguous runs
    out = nc.declare_dram_parameter("out", [2, rows, 2, HW], BF16,
                                    isOutput=True)

    xv = x.ap().rearrange("(p n) m -> p n m", p=P)                 # [128, npp, 81]
    ov = out.ap().rearrange("q (p n) j m -> p q n j m", p=P)

    with TileContext(nc) as tc:
        with tc.tile_pool(name="consts", bufs=1) as cpool, \
             tc.tile_pool(name="xs", bufs=1) as xpool, \
             tc.tile_pool(name="outs", bufs=6) as opool:
            # ---- const broadcast-load: first descriptors on the sync
            # HWDGE ring, so the 128 tiny step-0 packets drain before any
            # x packet and the grids are on-chip by ~9 us ----
            cb = cpool.tile([P, NCONST], F32)
            nc.sync.dma_start(cb, cst.ap().unsqueeze(0).broadcast_to([P, NCONST]))

            # ---- x loads, all queued now, all resident, alternating the
            # two HWDGE rings (no SWDGE: two queues total keeps the DMA
            # engines out of 3-way arbitration, and ring FIFO sequences
            # loads ahead of stores with zero engine idle) ----
            # first load chunk stays small so the const packets are not
            # starved behind 20.7 KB load packets at the engines
            LOADS = [32, 96, 128]
            xts = []  # (tile, row0, nrows)
            r0 = 0
            for i, nr in enumerate(LOADS):
                xt = xpool.tile([P, nr * HW], BF16, tag=f"x{i}", bufs=1,
                                name=f"xt{i}")
                eng = nc.scalar if i % 2 == 0 else nc.sync
                eng.dma_start(xt, xv[:, r0:r0 + nr, :])
                xts.append((xt, r0, nr))
                r0 += nr

            def xrows(n0, ns):  # [P, ns, 81] view of rows [n0, n0+ns)
                t, r0, nr = next(e for e in xts
                                 if e[1] <= n0 and n0 + ns <= e[1] + e[2])
                b = (n0 - r0) * HW
                return t[:, b:b + ns * HW].rearrange("p (n m) -> p n m", m=HW)

            xs_t = cb[:, 0:HW]
            ys_t = cb[:, HW:2 * HW]
            env_t = cb[:, 2 * HW:3 * HW]
            th_t = cb[:, 3 * HW:3 * HW + G]
            lm_t = cb[:, 3 * HW + G:3 * HW + 2 * G]

            def per_g(t):  # [128, G] -> [128, G, HW] step-0 view
                return t.unsqueeze(2).broadcast_to([P, G, HW])

            def over_g(ap):  # [128, 81] -> [128, G, 81] step-0 view
                return ap.unsqueeze(1).broadcast_to([P, G, HW])

            # ---- Gabor synthesis, replicated on all 128 partitions ----
            hpi = cpool.tile([P, 1], F32)
            nc.vector.memset(hpi, float(np.pi / 2))  # no DMA dep: runs early
            sin_t = cpool.tile([P, G], F32)
            nc.scalar.activation(sin_t, th_t, AF.Sin)                   # sin th
            cos_t = cpool.tile([P, G], F32)
            # cos th = sin(pi/2 - th); th in [0, 3pi/4] keeps the argument
            # inside ACT Sin's valid [-pi, pi]
            nc.scalar.activation(cos_t, th_t, AF.Sin, scale=-1.0, bias=hpi)

            xr = cpool.tile([P, G, HW], F32)
            t2 = cpool.tile([P, G, HW], F32)
            nc.vector.tensor_mul(xr, over_g(xs_t), per_g(cos_t))
            nc.vector.tensor_mul(t2, over_g(ys_t), per_g(sin_t))
            nc.vector.tensor_add(xr, xr, t2)                            # rotated x
            tt = cpool.tile([P, G, HW], F32)
            nc.vector.tensor_mul(tt, xr, per_g(lm_t))                   # t = xr*lam
            # range-reduce t via int32 round-trip (ACT Sin is only valid on
            # [-pi, pi]; DVE has no mod). Any nearby-integer shift k works:
            # cos(2pi t) = 1 - 2 sin^2(pi (t - k)).
            ti = cpool.tile([P, G, HW], I32)
            nc.vector.tensor_copy(ti, tt)
            tf = cpool.tile([P, G, HW], F32)
            nc.vector.tensor_copy(tf, ti)
            nc.vector.tensor_sub(tt, tt, tf)
            ss = cpool.tile([P, G, HW], F32)
            nc.scalar.activation(ss, tt, AF.Sin, scale=SIGMA)           # sin(pi m)
            gb = cpool.tile([P, G * HW], F32)
            gbg = gb.rearrange("p (g m) -> p g m", m=HW)
            nc.vector.tensor_mul(gbg, ss, ss)
            nc.vector.tensor_scalar(gb, gb, -2.0, 1.0, ALU.mult, ALU.add)  # cos
            gbb = cpool.tile([P, G * HW], BF16)
            gbbg = gbb.rearrange("p (g m) -> p g m", m=HW)
            nc.vector.tensor_mul(gbbg, gbg, over_g(env_t))  # * envelope, to bf16

            def gbv(g, ns):  # filter g broadcast over ns rows (step-0 view)
                return gbb[:, g * HW:(g + 1) * HW].unsqueeze(1).broadcast_to(
                    [P, ns, HW])

            # ---- streaming broadcast-multiply; two muls fill a g-pair-
            # interleaved tile, one store per (block, pair) alternating the
            # two HWDGE rings (SP and ACT) ----
            n0 = 0
            for i, ns in enumerate(BLOCKS):
                for q in range(2):
                    ot = opool.tile([P, NSMAX * 2 * HW], BF16, tag="o",
                                    name="ot")
                    otv = ot[:, 0:ns * 2 * HW].rearrange(
                        "p (n j m) -> p n j m", j=2, m=HW)
                    for j in range(2):
                        nc.vector.tensor_tensor(
                            otv[:, :, j, :], xrows(n0, ns), gbv(2 * q + j, ns),
                            ALU.mult)
                    eng = nc.sync if q == 0 else nc.scalar
                    eng.dma_start(ov[:, q, n0:n0 + ns], otv)
                n0 += ns
    nc.finalize()  # Bacc passes: wait legalization, reg alloc, act table loads
    return nc


def make_const_row(theta, lam):
    ys = np.arange(H, dtype=np.float32) - (H - 1) / 2.0
    xs = np.arange(W, dtype=np.float32) - (W - 1) / 2.0
    y, x = np.meshgrid(ys, xs, indexing="ij")
    env = np.exp(-(x ** 2 + y ** 2) / (2.0 * np.float32(SIGMA) ** 2))
    return np.concatenate(
        [v.reshape(-1) for v in (x, y, env)] + [theta, lam]
    ).astype(np.float32)  # [3 * 81 + 2 * G]


_NC = None
TRACE = False          # set True by the local test harness for NTFF timing
LAST_RESULT = None     # BassKernelResults of the most recent run


def kernel(x, theta, lam):
    global _NC
    if _NC is None:
        _NC = build_bass()
    x = np.asarray(x, dtype=np.float32)
    theta = np.asarray(theta, dtype=np.float32).reshape(G)
    lam = np.asarray(lam, dtype=np.float32).reshape(G)
    cst = make_const_row(theta, lam)
    xb = np.ascontiguousarray(x.astype(ml_dtypes.bfloat16))

    in_maps = []
    for m in range(N_CORES):
        shard = xb[m * CO_SH:(m + 1) * CO_SH].reshape(ROWS, HW)
        in_maps.append({"x": shard, "cst": cst})

    global LAST_RESULT
    LAST_RESULT = run_bass_kernel_spmd(
        _NC, in_maps, list(range(N_CORES)), trace=TRACE
    )
    res = LAST_RESULT.results

    out = np.empty((G, CO, CI, H, W), dtype=np.float32)
    for m in range(N_CORES):
        o = np.asarray(res[m]["out"]).astype(np.float32)
        o = o.reshape(2, ROWS, 2, HW).transpose(0, 2, 1, 3)  # -> [G, rows, hw]
        out[:, m * CO_SH:(m + 1) * CO_SH] = o.reshape(G, CO_SH, CI, H, W)
    return out.reshape(G * CO, CI, H, W)

